# revision 27
# baseline (speedup 1.0000x reference)
"""LocationSensitiveAttention Trainium2 kernel (Bass/Tile), data-parallel over 8 NeuronCores.

Reference computation (per batch b):
    pq = hidden @ W_query                        # (A,)
    pm = memory @ W_memory                       # (T, A)
    loc = conv1d(aw_cat, conv_w, 'same')         # (F, T)
    pl = loc^T @ W_loc                           # (T, A)
    energies = tanh(pm + pq + pl) @ v_w + v_b    # (T,)
    align = energies + mask * -1e25              # (T,)
    attention_weights = softmax(align)           # (T,)
    attention_output = align @ memory            # (E,)  (raw align, not softmax!)

Device strategy (per core, 8 batches each):
  - memory uploaded host-transposed (E-major, bf16): tiles [128=E-chunk, T] so
    the tensor engine contracts over E with W_memory chunks as stationary
    weights, producing (pm+pl)^T [A=128, T-tile] directly in PSUM (conv folded
    to one matmul via host im2col of aw_cat + host-fused (conv_w @ W_loc)).
  - tanh on the scalar engine with bias = pq column (pq from a small matmul).
  - energies via matmul with v_w replicated across 128 columns -> PSUM holds
    align broadcast across all 128 partitions; the mask row (+v_b) is added by
    a rank-1 matmul in the same accumulation group. One PSUM->SBUF copy feeds
    both the softmax row extraction and the attention_output weighted sum.
  - attention_output = per E-chunk fused multiply+reduce on the vector engine
    (scalar_tensor_tensor with accum_out) over the free (T) dimension.
  - softmax batched over the 8 batch rows on partitions.
"""

import sys

if "/opt/trn_rl_repo" not in sys.path:
    sys.path.insert(0, "/opt/trn_rl_repo")

from contextlib import ExitStack

import ml_dtypes
import numpy as np
from numpy.lib.stride_tricks import sliding_window_view

import concourse.bass as bass
import concourse.mybir as mybir
import concourse.tile as tile
from concourse import bacc, bass_utils

# Problem dims (hardcoded per contract)
B, T, E, R, A, F, K = 64, 2048, 512, 1024, 128, 32, 31
NCORES = 8
BPC = B // NCORES  # batches per core = 8
CK = 2 * K  # 62 im2col rows
EC = E // 128  # 4 E-chunks
TJ = T // 512  # 4 T-tiles of 512
F32 = mybir.dt.float32
F32R = mybir.dt.float32r
BF16 = mybir.dt.bfloat16
MEM_BF16 = True  # memory-tensor numerics: True=bf16 (fast), False=f32r (accurate)
MDT = BF16 if MEM_BF16 else F32R

_CACHE = {}


def _build_module():
    nc = bacc.Bacc("TRN2", target_bir_lowering=False, debug=False, num_devices=1)

    memT = nc.dram_tensor("memT", [BPC * 128, EC * T], MDT, kind="ExternalInput").ap()
    x62 = nc.dram_tensor("x62", [BPC * CK, T], MDT, kind="ExternalInput").ap()
    hT = nc.dram_tensor("hT", [128, (R // 128) * BPC], F32R, kind="ExternalInput").ap()
    maskf = nc.dram_tensor("maskf", [BPC, T], F32R, kind="ExternalInput").ap()
    wq = nc.dram_tensor("wq", [128, (R // 128) * A], F32R, kind="ExternalInput").ap()
    wmem = nc.dram_tensor("wmem", [128, EC * A], MDT, kind="ExternalInput").ap()
    wf62 = nc.dram_tensor("wf62", [CK, A], MDT, kind="ExternalInput").ap()
    vwrep = nc.dram_tensor("vwrep", [A, 128], MDT, kind="ExternalInput").ap()
    ident = nc.dram_tensor("ident", [128, 128], F32, kind="ExternalInput").ap()
    ones = nc.dram_tensor("ones", [1, 128], F32R, kind="ExternalInput").ap()

    outv = nc.dram_tensor("outv", [BPC * EC, 128], F32, kind="ExternalOutput").ap()
    attnw = nc.dram_tensor("attnw", [BPC, T], F32, kind="ExternalOutput").ap()

    Tanh = mybir.ActivationFunctionType.Tanh
    Exp = mybir.ActivationFunctionType.Exp

    with tile.TileContext(nc) as tc, ExitStack() as ctx:
        consts = ctx.enter_context(tc.tile_pool(name="consts", bufs=1))
        mem_pool = ctx.enter_context(tc.tile_pool(name="mem", bufs=6))
        x62_pool = ctx.enter_context(tc.tile_pool(name="x62", bufs=3))
        maskb_pool = ctx.enter_context(tc.tile_pool(name="maskb", bufs=2))
        bc_pool = ctx.enter_context(tc.tile_pool(name="bc", bufs=3))
        th_pool = ctx.enter_context(tc.tile_pool(name="th", bufs=3))
        small = ctx.enter_context(tc.tile_pool(name="small", bufs=1))
        ph_pool = ctx.enter_context(tc.tile_pool(name="ph", bufs=4, space="PSUM"))
        pbc_pool = ctx.enter_context(tc.tile_pool(name="pbc", bufs=2, space="PSUM"))
        pmisc_pool = ctx.enter_context(tc.tile_pool(name="pmisc", bufs=1, space="PSUM"))

        # ---- constants / params (pre-arranged on host for straight loads) ----
        wmem_sb = consts.tile([128, E], MDT)
        nc.scalar.dma_start(wmem_sb[:], wmem[:])
        wq_sb = consts.tile([128, R], F32R)
        nc.scalar.dma_start(wq_sb[:], wq[:])
        hT_sb = consts.tile([128, (R // 128) * BPC], F32R)
        nc.scalar.dma_start(hT_sb[:], hT[:])
        wf62_sb = consts.tile([CK, A], MDT)
        nc.gpsimd.dma_start(wf62_sb[:], wf62[:])
        vwrep_sb = consts.tile([A, 128], MDT)
        nc.gpsimd.dma_start(vwrep_sb[:], vwrep[:])
        ident_sb = consts.tile([128, 128], F32)
        nc.gpsimd.dma_start(ident_sb[:], ident[:])
        ones_row = consts.tile([1, 128], F32R)
        nc.gpsimd.dma_start(ones_row[:], ones[:])

        # ---- pq^T [A, BPC] ----
        ppq = pmisc_pool.tile([A, BPC], F32)
        for c in range(R // 128):
            nc.tensor.matmul(
                ppq[:],
                wq_sb[:, bass.ts(c, 128)],
                hT_sb[:, bass.ts(c, BPC)],
                start=(c == 0),
                stop=(c == R // 128 - 1),
            )
        pq_sb = small.tile([A, BPC], F32)
        nc.scalar.copy(pq_sb[:], ppq[:])

        aligns_sb = small.tile([BPC, T], MDT)
        outT_sb = small.tile([128, BPC * EC], F32)
        trash = small.tile([128, T], MDT)

        # ---- main per-batch pipeline ----
        for b in range(BPC):
            mb = mem_pool.tile([128, EC * T], MDT, tag="mem")
            nc.sync.dma_start(mb[0:64, :], memT[b * 128 : b * 128 + 64, :])
            nc.scalar.dma_start(mb[64:128, :], memT[b * 128 + 64 : (b + 1) * 128, :])
            chunk = lambda c: mb[:, c * T : (c + 1) * T]
            chunk_js = lambda c, j: mb[:, c * T + j * 512 : c * T + j * 512 + 512]
            xb = x62_pool.tile([CK, T], MDT)
            (nc.sync if b % 2 == 0 else nc.scalar).dma_start(xb[:], x62[b * CK : (b + 1) * CK, :])
            mkb = maskb_pool.tile([1, T], F32R)
            nc.gpsimd.dma_start(mkb[:], maskf[b : b + 1, :])

            bc_sb = bc_pool.tile([128, T], MDT)
            phs = [
                ph_pool.tile([128, 512], F32, tag="ph", name=f"ph{b}_{jj}")
                for jj in range(TJ)
            ]
            for c in range(EC):
                for j in range(TJ):
                    nc.tensor.matmul(
                        phs[j][:],
                        wmem_sb[:, bass.ts(c, 128)],
                        chunk_js(c, j),
                        start=(c == 0),
                        stop=False,
                    )
            ths = []
            for j in range(TJ):
                js = bass.ts(j, 512)
                nc.tensor.matmul(
                    phs[j][:], wf62_sb[:], xb[:, js], start=False, stop=True
                )
                th = th_pool.tile([A, 512], MDT, tag="th")
                nc.scalar.activation(th[:], phs[j][:], Tanh, bias=pq_sb[:, b : b + 1])
                ths.append(th)
            for j in range(TJ):
                js = bass.ts(j, 512)
                pbc = pbc_pool.tile([128, 512], F32, tag="pbc")
                nc.tensor.matmul(pbc[:], vwrep_sb[:], ths[j][:], start=True, stop=False)
                nc.tensor.matmul(
                    pbc[:], ones_row[:], mkb[0:1, js], start=False, stop=True
                )
                nc.scalar.copy(bc_sb[:, js], pbc[:])

            nc.gpsimd.dma_start(aligns_sb[b : b + 1, :], bc_sb[0:1, :])

            for c in range(EC):
                nc.vector.scalar_tensor_tensor(
                    out=trash[:],
                    in0=chunk(c),
                    scalar=1.0,
                    in1=bc_sb[:],
                    op0=mybir.AluOpType.mult,
                    op1=mybir.AluOpType.mult,
                    accum_out=outT_sb[:, b * EC + c : b * EC + c + 1],
                )

        # ---- softmax over T for all 8 batch rows ----
        nmax = small.tile([BPC, 1], F32)
        nc.vector.tensor_reduce(
            nmax[:], aligns_sb[:], mybir.AxisListType.X, mybir.AluOpType.max, negate=True
        )
        expt = small.tile([BPC, T], F32)
        nc.scalar.activation(expt[:], aligns_sb[:], Exp, bias=nmax[:, 0:1])
        ssum = small.tile([BPC, 1], F32)
        nc.vector.tensor_reduce(
            ssum[:], expt[:], mybir.AxisListType.X, mybir.AluOpType.add
        )
        rec = small.tile([BPC, 1], F32)
        nc.vector.reciprocal(rec[:], ssum[:])
        attnw_sb = small.tile([BPC, T], F32)
        nc.scalar.activation(
            attnw_sb[:], expt[:], mybir.ActivationFunctionType.Copy, scale=rec[:, 0:1]
        )
        nc.sync.dma_start(attnw[:], attnw_sb[:])

        # ---- attention_output: transpose [128, 32] -> [32, 128] and store ----
        pT = pmisc_pool.tile([BPC * EC, 128], F32)
        nc.tensor.transpose(pT[:], outT_sb[:], ident_sb[:])
        outrows = small.tile([BPC * EC, 128], F32)
        nc.scalar.copy(outrows[:], pT[:])
        nc.sync.dma_start(outv[:], outrows[:])

    nc.compile()
    return nc


def unpermute_out(outv_core):
    """outv rows are (b, c); element e = 4p + c."""
    return outv_core.reshape(BPC, EC, 128).transpose(0, 2, 1).reshape(BPC, E)


def _host_prep(inputs):
    """Full inputs -> per-core in_maps (host-side sharding + layout prep)."""
    memory = np.ascontiguousarray(np.asarray(inputs["memory"], dtype=np.float32))
    hidden = np.asarray(inputs["attention_hidden"], dtype=np.float32)
    awc = np.asarray(inputs["attention_weights_cat"], dtype=np.float32)
    mask = np.asarray(inputs["mask"])
    wq = np.ascontiguousarray(np.asarray(inputs["W_query"], dtype=np.float32))
    wmem = np.ascontiguousarray(np.asarray(inputs["W_memory"], dtype=np.float32))
    conv_w = np.asarray(inputs["conv_w"], dtype=np.float32)
    wloc = np.asarray(inputs["W_loc"], dtype=np.float32)
    vw = np.asarray(inputs["v_w"], dtype=np.float32)
    vb = np.asarray(inputs["v_b"], dtype=np.float32)

    mnp = ml_dtypes.bfloat16 if MEM_BF16 else np.float32

    # fused conv+location weights: [62, A]
    w62 = conv_w.transpose(1, 2, 0).reshape(CK, F)  # [(c,k), f]
    wf62 = np.ascontiguousarray(w62 @ wloc).astype(mnp)
    vwrep = np.ascontiguousarray(np.repeat(vw[:, None], 128, axis=1)).astype(mnp)
    ident = np.eye(128, dtype=np.float32)

    # wmem chunks (4-row packing): wmem_sb[p, c*A + a] = wmem[4p + c, a]
    wmem_p = np.ascontiguousarray(wmem.reshape(128, EC * A)).astype(mnp)
    # wq chunk c, partition p <- row 8p + c (permutation-invariant contraction)
    wq_p = np.ascontiguousarray(wq.reshape(128, 8, A).reshape(128, 8 * A))

    # im2col of aw_cat with 'same' zero padding: [B, 62, T]
    xp = np.pad(awc, ((0, 0), (0, 0), ((K - 1) // 2, (K - 1) // 2)))
    sw = sliding_window_view(xp, T, axis=2)  # [B, 2, 31, T]
    x62_full = sw.reshape(B, CK, T)

    maskf_full = mask.astype(np.float32) * np.float32(-1e25) + np.float32(vb[0])

    in_maps = []
    for core in range(NCORES):
        bs = slice(core * BPC, (core + 1) * BPC)
        hTp = np.ascontiguousarray(
            hidden[bs].T.reshape(128, 8, BPC).reshape(128, 8 * BPC)
        )
        in_maps.append(
            {
                "memT": np.ascontiguousarray(memory[bs].transpose(0, 2, 1))
                .reshape(BPC * 128, EC * T)
                .astype(mnp),
                "x62": np.ascontiguousarray(x62_full[bs])
                .reshape(BPC * CK, T)
                .astype(mnp),
                "hT": hTp,
                "maskf": np.ascontiguousarray(maskf_full[bs]),
                "wq": wq_p,
                "wmem": wmem_p,
                "wf62": wf62,
                "vwrep": vwrep,
                "ident": ident,
                "ones": np.ones((1, 128), np.float32),
            }
        )
    return in_maps


def get_module():
    if "nc" not in _CACHE:
        _CACHE["nc"] = _build_module()
    return _CACHE["nc"]


def run(inputs, trace=False):
    nc = get_module()
    in_maps = _host_prep(inputs)
    res = bass_utils.run_bass_kernel_spmd(
        nc, in_maps, core_ids=list(range(NCORES)), trace=trace
    )
    att_out = np.concatenate(
        [unpermute_out(res.results[c]["outv"]) for c in range(NCORES)], axis=0
    )
    att_w = np.concatenate([res.results[c]["attnw"] for c in range(NCORES)], axis=0)
    return (att_out, att_w), res


def kernel(**inputs):
    outs, _ = run(inputs, trace=False)
    return outs


# revision 28
# speedup vs baseline: 1.2459x; 1.2459x over previous
"""LocationSensitiveAttention Trainium2 kernel (Bass/Tile), data-parallel over 8 NeuronCores.

Reference computation (per batch b):
    pq = hidden @ W_query                        # (A,)
    pm = memory @ W_memory                       # (T, A)
    loc = conv1d(aw_cat, conv_w, 'same')         # (F, T)
    pl = loc^T @ W_loc                           # (T, A)
    energies = tanh(pm + pq + pl) @ v_w + v_b    # (T,)
    align = energies + mask * -1e25              # (T,)
    attention_weights = softmax(align)           # (T,)
    attention_output = align @ memory            # (E,)  (raw align, not softmax!)

Device strategy (per core, 8 batches each):
  - memory uploaded host-transposed (E-major, bf16): tiles [128=E-chunk, T] so
    the tensor engine contracts over E with W_memory chunks as stationary
    weights, producing (pm+pl)^T [A=128, T-tile] directly in PSUM (conv folded
    to one matmul via host im2col of aw_cat + host-fused (conv_w @ W_loc)).
  - tanh on the scalar engine with bias = pq column (pq from a small matmul).
  - energies via matmul with v_w replicated across 128 columns -> PSUM holds
    align broadcast across all 128 partitions; the mask row (+v_b) is added by
    a rank-1 matmul in the same accumulation group. One PSUM->SBUF copy feeds
    both the softmax row extraction and the attention_output weighted sum.
  - attention_output = per E-chunk fused multiply+reduce on the vector engine
    (scalar_tensor_tensor with accum_out) over the free (T) dimension.
  - softmax batched over the 8 batch rows on partitions.
"""

import sys

if "/opt/trn_rl_repo" not in sys.path:
    sys.path.insert(0, "/opt/trn_rl_repo")

from contextlib import ExitStack

import ml_dtypes
import numpy as np
from numpy.lib.stride_tricks import sliding_window_view

import concourse.bass as bass
import concourse.mybir as mybir
import concourse.tile as tile
from concourse import bacc, bass_utils

# Problem dims (hardcoded per contract)
B, T, E, R, A, F, K = 64, 2048, 512, 1024, 128, 32, 31
NCORES = 8
BPC = B // NCORES  # batches per core = 8
CK = 2 * K  # 62 im2col rows
EC = E // 128  # 4 E-chunks
TJ = T // 512  # 4 T-tiles of 512
F32 = mybir.dt.float32
F32R = mybir.dt.float32r
BF16 = mybir.dt.bfloat16
MEM_BF16 = True  # memory-tensor numerics: True=bf16 (fast), False=f32r (accurate)
MDT = BF16 if MEM_BF16 else F32R

_CACHE = {}


def _build_module():
    nc = bacc.Bacc("TRN2", target_bir_lowering=False, debug=False, num_devices=1)

    memT = nc.dram_tensor("memT", [BPC * 128, EC * T], MDT, kind="ExternalInput").ap()
    x62 = nc.dram_tensor("x62", [BPC * CK, T], MDT, kind="ExternalInput").ap()
    hT = nc.dram_tensor("hT", [128, (R // 128) * BPC], F32R, kind="ExternalInput").ap()
    maskf = nc.dram_tensor("maskf", [BPC, T], F32R, kind="ExternalInput").ap()
    wq = nc.dram_tensor("wq", [128, (R // 128) * A], F32R, kind="ExternalInput").ap()
    wmem = nc.dram_tensor("wmem", [128, EC * A], MDT, kind="ExternalInput").ap()
    wf62 = nc.dram_tensor("wf62", [CK, A], MDT, kind="ExternalInput").ap()
    vwrep = nc.dram_tensor("vwrep", [A, 128], MDT, kind="ExternalInput").ap()
    ident = nc.dram_tensor("ident", [128, 128], F32, kind="ExternalInput").ap()
    ones = nc.dram_tensor("ones", [1, 128], F32R, kind="ExternalInput").ap()

    outv = nc.dram_tensor("outv", [BPC * EC, 128], F32, kind="ExternalOutput").ap()
    attnw = nc.dram_tensor("attnw", [BPC, T], F32, kind="ExternalOutput").ap()

    Tanh = mybir.ActivationFunctionType.Tanh
    Exp = mybir.ActivationFunctionType.Exp

    with tile.TileContext(nc) as tc, ExitStack() as ctx:
        consts = ctx.enter_context(tc.tile_pool(name="consts", bufs=1))
        mem_pool = ctx.enter_context(tc.tile_pool(name="mem", bufs=12))
        x62_pool = ctx.enter_context(tc.tile_pool(name="x62", bufs=3))
        maskb_pool = ctx.enter_context(tc.tile_pool(name="maskb", bufs=2))
        bc_pool = ctx.enter_context(tc.tile_pool(name="bc", bufs=3))
        th_pool = ctx.enter_context(tc.tile_pool(name="th", bufs=3))
        small = ctx.enter_context(tc.tile_pool(name="small", bufs=1))
        ph_pool = ctx.enter_context(tc.tile_pool(name="ph", bufs=4, space="PSUM"))
        pbc_pool = ctx.enter_context(tc.tile_pool(name="pbc", bufs=2, space="PSUM"))
        pmisc_pool = ctx.enter_context(tc.tile_pool(name="pmisc", bufs=1, space="PSUM"))

        # ---- constants / params (pre-arranged on host for straight loads) ----
        wmem_sb = consts.tile([128, E], MDT)
        nc.scalar.dma_start(wmem_sb[:], wmem[:])
        wq_sb = consts.tile([128, R], F32R)
        nc.scalar.dma_start(wq_sb[:], wq[:])
        hT_sb = consts.tile([128, (R // 128) * BPC], F32R)
        nc.scalar.dma_start(hT_sb[:], hT[:])
        wf62_sb = consts.tile([CK, A], MDT)
        nc.gpsimd.dma_start(wf62_sb[:], wf62[:])
        vwrep_sb = consts.tile([A, 128], MDT)
        nc.gpsimd.dma_start(vwrep_sb[:], vwrep[:])
        ident_sb = consts.tile([128, 128], F32)
        nc.gpsimd.dma_start(ident_sb[:], ident[:])
        ones_row = consts.tile([1, 128], F32R)
        nc.gpsimd.dma_start(ones_row[:], ones[:])

        # ---- pq^T [A, BPC] ----
        ppq = pmisc_pool.tile([A, BPC], F32)
        for c in range(R // 128):
            nc.tensor.matmul(
                ppq[:],
                wq_sb[:, bass.ts(c, 128)],
                hT_sb[:, bass.ts(c, BPC)],
                start=(c == 0),
                stop=(c == R // 128 - 1),
            )
        pq_sb = small.tile([A, BPC], F32)
        nc.scalar.copy(pq_sb[:], ppq[:])

        aligns_sb = small.tile([BPC, T], MDT)
        outT_sb = small.tile([128, BPC * EC], F32)
        trash = small.tile([128, T], MDT)

        # ---- main per-batch pipeline ----
        for b in range(BPC):
            pairs = []
            for k in range(2):
                m = mem_pool.tile([128, 2 * T], MDT, tag="mem", name=f"mem{b}_{k}")
                dma_eng = nc.sync if k == 0 else nc.scalar
                dma_eng.dma_start(
                    m[:], memT[b * 128 : (b + 1) * 128, k * 2 * T : (k + 1) * 2 * T]
                )
                pairs.append(m)
            chunk = lambda c: pairs[c // 2][:, (c % 2) * T : (c % 2) * T + T]
            chunk_js = lambda c, j: pairs[c // 2][
                :, (c % 2) * T + j * 512 : (c % 2) * T + j * 512 + 512
            ]
            xb = x62_pool.tile([CK, T], MDT)
            (nc.sync if b % 2 == 0 else nc.scalar).dma_start(xb[:], x62[b * CK : (b + 1) * CK, :])
            mkb = maskb_pool.tile([1, T], F32R)
            nc.gpsimd.dma_start(mkb[:], maskf[b : b + 1, :])

            bc_sb = bc_pool.tile([128, T], MDT)
            phs = [
                ph_pool.tile([128, 512], F32, tag="ph", name=f"ph{b}_{jj}")
                for jj in range(TJ)
            ]
            for c in range(EC):
                for j in range(TJ):
                    nc.tensor.matmul(
                        phs[j][:],
                        wmem_sb[:, bass.ts(c, 128)],
                        chunk_js(c, j),
                        start=(c == 0),
                        stop=False,
                    )
            ths = []
            for j in range(TJ):
                js = bass.ts(j, 512)
                nc.tensor.matmul(
                    phs[j][:], wf62_sb[:], xb[:, js], start=False, stop=True
                )
                th = th_pool.tile([A, 512], MDT, tag="th")
                nc.scalar.activation(th[:], phs[j][:], Tanh, bias=pq_sb[:, b : b + 1])
                ths.append(th)
            for j in range(TJ):
                js = bass.ts(j, 512)
                pbc = pbc_pool.tile([128, 512], F32, tag="pbc")
                nc.tensor.matmul(pbc[:], vwrep_sb[:], ths[j][:], start=True, stop=False)
                nc.tensor.matmul(
                    pbc[:], ones_row[:], mkb[0:1, js], start=False, stop=True
                )
                nc.scalar.copy(bc_sb[:, js], pbc[:])

            nc.gpsimd.dma_start(aligns_sb[b : b + 1, :], bc_sb[0:1, :])

            for c in range(EC):
                nc.vector.scalar_tensor_tensor(
                    out=trash[:],
                    in0=chunk(c),
                    scalar=1.0,
                    in1=bc_sb[:],
                    op0=mybir.AluOpType.mult,
                    op1=mybir.AluOpType.mult,
                    accum_out=outT_sb[:, b * EC + c : b * EC + c + 1],
                )

        # ---- softmax over T for all 8 batch rows ----
        nmax = small.tile([BPC, 1], F32)
        nc.vector.tensor_reduce(
            nmax[:], aligns_sb[:], mybir.AxisListType.X, mybir.AluOpType.max, negate=True
        )
        expt = small.tile([BPC, T], F32)
        nc.scalar.activation(expt[:], aligns_sb[:], Exp, bias=nmax[:, 0:1])
        ssum = small.tile([BPC, 1], F32)
        nc.vector.tensor_reduce(
            ssum[:], expt[:], mybir.AxisListType.X, mybir.AluOpType.add
        )
        rec = small.tile([BPC, 1], F32)
        nc.vector.reciprocal(rec[:], ssum[:])
        attnw_sb = small.tile([BPC, T], F32)
        nc.scalar.activation(
            attnw_sb[:], expt[:], mybir.ActivationFunctionType.Copy, scale=rec[:, 0:1]
        )
        nc.sync.dma_start(attnw[:], attnw_sb[:])

        # ---- attention_output: transpose [128, 32] -> [32, 128] and store ----
        pT = pmisc_pool.tile([BPC * EC, 128], F32)
        nc.tensor.transpose(pT[:], outT_sb[:], ident_sb[:])
        outrows = small.tile([BPC * EC, 128], F32)
        nc.scalar.copy(outrows[:], pT[:])
        nc.sync.dma_start(outv[:], outrows[:])

    nc.compile()
    return nc


def unpermute_out(outv_core):
    """outv rows are (b, c); element e = 256*(c//2) + 2p + c%2."""
    return (
        outv_core.reshape(BPC, 2, 2, 128).transpose(0, 1, 3, 2).reshape(BPC, E)
    )


def _host_prep(inputs):
    """Full inputs -> per-core in_maps (host-side sharding + layout prep)."""
    memory = np.ascontiguousarray(np.asarray(inputs["memory"], dtype=np.float32))
    hidden = np.asarray(inputs["attention_hidden"], dtype=np.float32)
    awc = np.asarray(inputs["attention_weights_cat"], dtype=np.float32)
    mask = np.asarray(inputs["mask"])
    wq = np.ascontiguousarray(np.asarray(inputs["W_query"], dtype=np.float32))
    wmem = np.ascontiguousarray(np.asarray(inputs["W_memory"], dtype=np.float32))
    conv_w = np.asarray(inputs["conv_w"], dtype=np.float32)
    wloc = np.asarray(inputs["W_loc"], dtype=np.float32)
    vw = np.asarray(inputs["v_w"], dtype=np.float32)
    vb = np.asarray(inputs["v_b"], dtype=np.float32)

    mnp = ml_dtypes.bfloat16 if MEM_BF16 else np.float32

    # fused conv+location weights: [62, A]
    w62 = conv_w.transpose(1, 2, 0).reshape(CK, F)  # [(c,k), f]
    wf62 = np.ascontiguousarray(w62 @ wloc).astype(mnp)
    vwrep = np.ascontiguousarray(np.repeat(vw[:, None], 128, axis=1)).astype(mnp)
    ident = np.eye(128, dtype=np.float32)

    # wmem chunks (pair-packed): wmem_sb[p, c*A + a] = wmem[256*(c//2) + 2p + c%2, a]
    wmem_p = np.ascontiguousarray(
        wmem.reshape(2, 128, 2, A).transpose(1, 0, 2, 3).reshape(128, EC * A)
    ).astype(mnp)
    # wq chunk c, partition p <- row 8p + c (permutation-invariant contraction)
    wq_p = np.ascontiguousarray(wq.reshape(128, 8, A).reshape(128, 8 * A))

    # im2col of aw_cat with 'same' zero padding: [B, 62, T]
    xp = np.pad(awc, ((0, 0), (0, 0), ((K - 1) // 2, (K - 1) // 2)))
    sw = sliding_window_view(xp, T, axis=2)  # [B, 2, 31, T]
    x62_full = sw.reshape(B, CK, T)

    maskf_full = mask.astype(np.float32) * np.float32(-1e25) + np.float32(vb[0])

    in_maps = []
    for core in range(NCORES):
        bs = slice(core * BPC, (core + 1) * BPC)
        hTp = np.ascontiguousarray(
            hidden[bs].T.reshape(128, 8, BPC).reshape(128, 8 * BPC)
        )
        in_maps.append(
            {
                "memT": np.ascontiguousarray(
                    memory[bs]
                    .transpose(0, 2, 1)
                    .reshape(BPC, 2, 128, 2, T)
                    .transpose(0, 2, 1, 3, 4)
                )
                .reshape(BPC * 128, EC * T)
                .astype(mnp),
                "x62": np.ascontiguousarray(x62_full[bs])
                .reshape(BPC * CK, T)
                .astype(mnp),
                "hT": hTp,
                "maskf": np.ascontiguousarray(maskf_full[bs]),
                "wq": wq_p,
                "wmem": wmem_p,
                "wf62": wf62,
                "vwrep": vwrep,
                "ident": ident,
                "ones": np.ones((1, 128), np.float32),
            }
        )
    return in_maps


def get_module():
    if "nc" not in _CACHE:
        _CACHE["nc"] = _build_module()
    return _CACHE["nc"]


def run(inputs, trace=False):
    nc = get_module()
    in_maps = _host_prep(inputs)
    res = bass_utils.run_bass_kernel_spmd(
        nc, in_maps, core_ids=list(range(NCORES)), trace=trace
    )
    att_out = np.concatenate(
        [unpermute_out(res.results[c]["outv"]) for c in range(NCORES)], axis=0
    )
    att_w = np.concatenate([res.results[c]["attnw"] for c in range(NCORES)], axis=0)
    return (att_out, att_w), res


def kernel(**inputs):
    outs, _ = run(inputs, trace=False)
    return outs


# revision 29
# speedup vs baseline: 1.3245x; 1.0631x over previous
"""LocationSensitiveAttention Trainium2 kernel (Bass/Tile), data-parallel over 8 NeuronCores.

Reference computation (per batch b):
    pq = hidden @ W_query                        # (A,)
    pm = memory @ W_memory                       # (T, A)
    loc = conv1d(aw_cat, conv_w, 'same')         # (F, T)
    pl = loc^T @ W_loc                           # (T, A)
    energies = tanh(pm + pq + pl) @ v_w + v_b    # (T,)
    align = energies + mask * -1e25              # (T,)
    attention_weights = softmax(align)           # (T,)
    attention_output = align @ memory            # (E,)  (raw align, not softmax!)

Device strategy (per core, 8 batches each):
  - memory uploaded host-transposed (E-major, bf16): tiles [128=E-chunk, T] so
    the tensor engine contracts over E with W_memory chunks as stationary
    weights, producing (pm+pl)^T [A=128, T-tile] directly in PSUM (conv folded
    to one matmul via host im2col of aw_cat + host-fused (conv_w @ W_loc)).
  - tanh on the scalar engine with bias = pq column (pq from a small matmul).
  - energies via matmul with v_w replicated across 128 columns -> PSUM holds
    align broadcast across all 128 partitions; the mask row (+v_b) is added by
    a rank-1 matmul in the same accumulation group. One PSUM->SBUF copy feeds
    both the softmax row extraction and the attention_output weighted sum.
  - attention_output = per E-chunk fused multiply+reduce on the vector engine
    (scalar_tensor_tensor with accum_out) over the free (T) dimension.
  - softmax batched over the 8 batch rows on partitions.
"""

import sys

if "/opt/trn_rl_repo" not in sys.path:
    sys.path.insert(0, "/opt/trn_rl_repo")

from contextlib import ExitStack

import ml_dtypes
import numpy as np
from numpy.lib.stride_tricks import sliding_window_view

import concourse.bass as bass
import concourse.mybir as mybir
import concourse.tile as tile
from concourse import bacc, bass_utils

# Problem dims (hardcoded per contract)
B, T, E, R, A, F, K = 64, 2048, 512, 1024, 128, 32, 31
NCORES = 8
BPC = B // NCORES  # batches per core = 8
CK = 2 * K  # 62 im2col rows
EC = E // 128  # 4 E-chunks
TJ = T // 512  # 4 T-tiles of 512
F32 = mybir.dt.float32
F32R = mybir.dt.float32r
BF16 = mybir.dt.bfloat16
MEM_BF16 = True  # memory-tensor numerics: True=bf16 (fast), False=f32r (accurate)
MDT = BF16 if MEM_BF16 else F32R

_CACHE = {}


def _build_module():
    nc = bacc.Bacc("TRN2", target_bir_lowering=False, debug=False, num_devices=1)

    memT = nc.dram_tensor("memT", [BPC * 128, EC * T], MDT, kind="ExternalInput").ap()
    x62 = nc.dram_tensor("x62", [BPC * CK, T], MDT, kind="ExternalInput").ap()
    hT = nc.dram_tensor("hT", [128, (R // 128) * BPC], F32R, kind="ExternalInput").ap()
    maskf = nc.dram_tensor("maskf", [BPC, T], F32R, kind="ExternalInput").ap()
    wq = nc.dram_tensor("wq", [128, (R // 128) * A], F32R, kind="ExternalInput").ap()
    wmem = nc.dram_tensor("wmem", [128, EC * A], MDT, kind="ExternalInput").ap()
    wf62 = nc.dram_tensor("wf62", [CK, A], MDT, kind="ExternalInput").ap()
    vwrep = nc.dram_tensor("vwrep", [A, 128], MDT, kind="ExternalInput").ap()
    ident = nc.dram_tensor("ident", [128, 128], F32, kind="ExternalInput").ap()
    ones = nc.dram_tensor("ones", [1, 128], F32R, kind="ExternalInput").ap()

    outv = nc.dram_tensor("outv", [BPC * EC, 128], F32, kind="ExternalOutput").ap()
    attnw = nc.dram_tensor("attnw", [BPC, T], F32, kind="ExternalOutput").ap()

    Tanh = mybir.ActivationFunctionType.Tanh
    Exp = mybir.ActivationFunctionType.Exp

    with tile.TileContext(nc) as tc, ExitStack() as ctx:
        consts = ctx.enter_context(tc.tile_pool(name="consts", bufs=1))
        mem_pool = ctx.enter_context(tc.tile_pool(name="mem", bufs=12))
        x62_pool = ctx.enter_context(tc.tile_pool(name="x62", bufs=3))
        maskb_pool = ctx.enter_context(tc.tile_pool(name="maskb", bufs=2))
        bc_pool = ctx.enter_context(tc.tile_pool(name="bc", bufs=3))
        th_pool = ctx.enter_context(tc.tile_pool(name="th", bufs=3))
        small = ctx.enter_context(tc.tile_pool(name="small", bufs=1))
        ph_pool = ctx.enter_context(tc.tile_pool(name="ph", bufs=4, space="PSUM"))
        pbc_pool = ctx.enter_context(tc.tile_pool(name="pbc", bufs=2, space="PSUM"))
        pmisc_pool = ctx.enter_context(tc.tile_pool(name="pmisc", bufs=1, space="PSUM"))

        # ---- constants / params (pre-arranged on host for straight loads) ----
        wmem_sb = consts.tile([128, E], MDT)
        nc.scalar.dma_start(wmem_sb[:], wmem[:])
        wq_sb = consts.tile([128, R], F32R)
        nc.scalar.dma_start(wq_sb[:], wq[:])
        hT_sb = consts.tile([128, (R // 128) * BPC], F32R)
        nc.scalar.dma_start(hT_sb[:], hT[:])
        wf62_sb = consts.tile([CK, A], MDT)
        nc.gpsimd.dma_start(wf62_sb[:], wf62[:])
        vwrep_sb = consts.tile([A, 128], MDT)
        nc.gpsimd.dma_start(vwrep_sb[:], vwrep[:])
        ident_sb = consts.tile([128, 128], F32)
        nc.gpsimd.dma_start(ident_sb[:], ident[:])
        ones_row = consts.tile([1, 128], F32R)
        nc.gpsimd.dma_start(ones_row[:], ones[:])

        # ---- pq^T [A, BPC] ----
        ppq = pmisc_pool.tile([A, BPC], F32)
        for c in range(R // 128):
            nc.tensor.matmul(
                ppq[:],
                wq_sb[:, bass.ts(c, 128)],
                hT_sb[:, bass.ts(c, BPC)],
                start=(c == 0),
                stop=(c == R // 128 - 1),
            )
        pq_sb = small.tile([A, BPC], F32)
        nc.scalar.copy(pq_sb[:], ppq[:])

        aligns_sb = small.tile([BPC, T], MDT)
        outT_sb = small.tile([128, BPC * EC], F32)
        trash = small.tile([128, T], MDT)

        # ---- main per-batch pipeline ----
        for b in range(BPC):
            pairs = []
            for k in range(2):
                m = mem_pool.tile([128, 2 * T], MDT, tag="mem", name=f"mem{b}_{k}")
                dma_eng = nc.sync if k == 0 else nc.scalar
                dma_eng.dma_start(
                    m[:], memT[b * 128 : (b + 1) * 128, k * 2 * T : (k + 1) * 2 * T]
                )
                pairs.append(m)
            chunk = lambda c: pairs[c // 2][:, (c % 2) * T : (c % 2) * T + T]
            chunk_js = lambda c, j: pairs[c // 2][
                :, (c % 2) * T + j * 512 : (c % 2) * T + j * 512 + 512
            ]
            xb = x62_pool.tile([CK, T], MDT)
            nc.sync.dma_start(xb[:], x62[b * CK : (b + 1) * CK, :])
            mkb = maskb_pool.tile([1, T], F32R)
            nc.gpsimd.dma_start(mkb[:], maskf[b : b + 1, :])

            bc_sb = bc_pool.tile([128, T], MDT)
            phs = [
                ph_pool.tile([128, 512], F32, tag="ph", name=f"ph{b}_{jj}")
                for jj in range(TJ)
            ]
            for c in range(EC):
                for j in range(TJ):
                    nc.tensor.matmul(
                        phs[j][:],
                        wmem_sb[:, bass.ts(c, 128)],
                        chunk_js(c, j),
                        start=(c == 0),
                        stop=False,
                    )
            ths = []
            for j in range(TJ):
                js = bass.ts(j, 512)
                nc.tensor.matmul(
                    phs[j][:], wf62_sb[:], xb[:, js], start=False, stop=True
                )
                th = th_pool.tile([A, 512], MDT, tag="th")
                nc.scalar.activation(th[:], phs[j][:], Tanh, bias=pq_sb[:, b : b + 1])
                ths.append(th)
            for j in range(TJ):
                js = bass.ts(j, 512)
                pbc = pbc_pool.tile([128, 512], F32, tag="pbc")
                nc.tensor.matmul(pbc[:], vwrep_sb[:], ths[j][:], start=True, stop=False)
                nc.tensor.matmul(
                    pbc[:], ones_row[:], mkb[0:1, js], start=False, stop=True
                )
                nc.scalar.copy(bc_sb[:, js], pbc[:])

            nc.gpsimd.dma_start(aligns_sb[b : b + 1, :], bc_sb[0:1, :])

            for c in range(EC):
                nc.vector.scalar_tensor_tensor(
                    out=trash[:],
                    in0=chunk(c),
                    scalar=1.0,
                    in1=bc_sb[:],
                    op0=mybir.AluOpType.mult,
                    op1=mybir.AluOpType.mult,
                    accum_out=outT_sb[:, b * EC + c : b * EC + c + 1],
                )

        # ---- softmax over T for all 8 batch rows ----
        nmax = small.tile([BPC, 1], F32)
        nc.vector.tensor_reduce(
            nmax[:], aligns_sb[:], mybir.AxisListType.X, mybir.AluOpType.max, negate=True
        )
        expt = small.tile([BPC, T], F32)
        nc.scalar.activation(expt[:], aligns_sb[:], Exp, bias=nmax[:, 0:1])
        ssum = small.tile([BPC, 1], F32)
        nc.vector.tensor_reduce(
            ssum[:], expt[:], mybir.AxisListType.X, mybir.AluOpType.add
        )
        rec = small.tile([BPC, 1], F32)
        nc.vector.reciprocal(rec[:], ssum[:])
        attnw_sb = small.tile([BPC, T], F32)
        nc.scalar.activation(
            attnw_sb[:], expt[:], mybir.ActivationFunctionType.Copy, scale=rec[:, 0:1]
        )
        nc.sync.dma_start(attnw[:], attnw_sb[:])

        # ---- attention_output: transpose [128, 32] -> [32, 128] and store ----
        pT = pmisc_pool.tile([BPC * EC, 128], F32)
        nc.tensor.transpose(pT[:], outT_sb[:], ident_sb[:])
        outrows = small.tile([BPC * EC, 128], F32)
        nc.scalar.copy(outrows[:], pT[:])
        nc.sync.dma_start(outv[:], outrows[:])

    nc.compile()
    return nc


def unpermute_out(outv_core):
    """outv rows are (b, c); element e = 256*(c//2) + 2p + c%2."""
    return (
        outv_core.reshape(BPC, 2, 2, 128).transpose(0, 1, 3, 2).reshape(BPC, E)
    )


def _host_prep(inputs):
    """Full inputs -> per-core in_maps (host-side sharding + layout prep)."""
    memory = np.ascontiguousarray(np.asarray(inputs["memory"], dtype=np.float32))
    hidden = np.asarray(inputs["attention_hidden"], dtype=np.float32)
    awc = np.asarray(inputs["attention_weights_cat"], dtype=np.float32)
    mask = np.asarray(inputs["mask"])
    wq = np.ascontiguousarray(np.asarray(inputs["W_query"], dtype=np.float32))
    wmem = np.ascontiguousarray(np.asarray(inputs["W_memory"], dtype=np.float32))
    conv_w = np.asarray(inputs["conv_w"], dtype=np.float32)
    wloc = np.asarray(inputs["W_loc"], dtype=np.float32)
    vw = np.asarray(inputs["v_w"], dtype=np.float32)
    vb = np.asarray(inputs["v_b"], dtype=np.float32)

    mnp = ml_dtypes.bfloat16 if MEM_BF16 else np.float32

    # fused conv+location weights: [62, A]
    w62 = conv_w.transpose(1, 2, 0).reshape(CK, F)  # [(c,k), f]
    wf62 = np.ascontiguousarray(w62 @ wloc).astype(mnp)
    vwrep = np.ascontiguousarray(np.repeat(vw[:, None], 128, axis=1)).astype(mnp)
    ident = np.eye(128, dtype=np.float32)

    # wmem chunks (pair-packed): wmem_sb[p, c*A + a] = wmem[256*(c//2) + 2p + c%2, a]
    wmem_p = np.ascontiguousarray(
        wmem.reshape(2, 128, 2, A).transpose(1, 0, 2, 3).reshape(128, EC * A)
    ).astype(mnp)
    # wq chunk c, partition p <- row 8p + c (permutation-invariant contraction)
    wq_p = np.ascontiguousarray(wq.reshape(128, 8, A).reshape(128, 8 * A))

    # im2col of aw_cat with 'same' zero padding: [B, 62, T]
    xp = np.pad(awc, ((0, 0), (0, 0), ((K - 1) // 2, (K - 1) // 2)))
    sw = sliding_window_view(xp, T, axis=2)  # [B, 2, 31, T]
    x62_full = sw.reshape(B, CK, T)

    maskf_full = mask.astype(np.float32) * np.float32(-1e25) + np.float32(vb[0])

    in_maps = []
    for core in range(NCORES):
        bs = slice(core * BPC, (core + 1) * BPC)
        hTp = np.ascontiguousarray(
            hidden[bs].T.reshape(128, 8, BPC).reshape(128, 8 * BPC)
        )
        in_maps.append(
            {
                "memT": np.ascontiguousarray(
                    memory[bs]
                    .transpose(0, 2, 1)
                    .reshape(BPC, 2, 128, 2, T)
                    .transpose(0, 2, 1, 3, 4)
                )
                .reshape(BPC * 128, EC * T)
                .astype(mnp),
                "x62": np.ascontiguousarray(x62_full[bs])
                .reshape(BPC * CK, T)
                .astype(mnp),
                "hT": hTp,
                "maskf": np.ascontiguousarray(maskf_full[bs]),
                "wq": wq_p,
                "wmem": wmem_p,
                "wf62": wf62,
                "vwrep": vwrep,
                "ident": ident,
                "ones": np.ones((1, 128), np.float32),
            }
        )
    return in_maps


def get_module():
    if "nc" not in _CACHE:
        _CACHE["nc"] = _build_module()
    return _CACHE["nc"]


def run(inputs, trace=False):
    nc = get_module()
    in_maps = _host_prep(inputs)
    res = bass_utils.run_bass_kernel_spmd(
        nc, in_maps, core_ids=list(range(NCORES)), trace=trace
    )
    att_out = np.concatenate(
        [unpermute_out(res.results[c]["outv"]) for c in range(NCORES)], axis=0
    )
    att_w = np.concatenate([res.results[c]["attnw"] for c in range(NCORES)], axis=0)
    return (att_out, att_w), res


def kernel(**inputs):
    outs, _ = run(inputs, trace=False)
    return outs
